# revision 9
# baseline (speedup 1.0000x reference)
"""APPNP GNN forward on 8 Trainium2 NeuronCores (Bass/Tile, SPMD).

Strategy (all 8 cores share one instruction stream; per-core data differs):
  - nodes sharded 12500/core; MLP data-parallel; z fp16 [128, NCHK*64] SBUF
  - z mirrored to HBM as two tables (chunks 0-47 / 48-97), 256B rows,
    double-buffered by step parity, AllGather'd separately so AG of half 0
    issues mid-step and overlaps compute / next-step gathers
  - edges partitioned by dst core; per (dst chunk, region) runs 32-aligned,
    tiles of 128 slots span chunk boundaries (matmuls partition-sliced at
    32/64/128 grid positions); region = (table, 4-core block)
  - slots gathered via dma_gather (int16 block-local row ids, 256B rows),
    one queue per region
  - S matrices ([128 slots, 128 dcol] fp16) built ON-CHIP per tile by one
    DVE tensor_scalar (iota == dcol) * w from resident per-tile dcol/w cols
  - aggregation via TensorE into per-chunk PSUM [128, 64]; alpha*h and the
    self-loop term are applied by DVE at copy-out (no init matmuls)
  - final iteration computes log_softmax on-chip from PSUM, fp32 out
"""
import sys, os, types

sys.path.insert(0, "/opt/trn_rl_repo")
import numpy as np

N = 100000
NCORE = 8
NPC = N // NCORE             # 12500
CH = 128
NCHK = (NPC + CH - 1) // CH  # 98
SPLIT = 48                   # chunks 0-47 -> table 0, 48-97 -> table 1
NR = (SPLIT, NCHK - SPLIT)   # rows-per-partition per table (48, 50)
GSIZE = 8
ALPHA = 0.1
MAX_CALL_TILES = 12
K_ITERS = 10
F_IN = 128
HID = 256
CLS = 47
NREG = 4                     # (table 0/1) x (4-core block 0/1)
SINGLE_PACKET = False
SEG_ALIGN = 32               # 32: K-sliced matmuls; 128: full-tile only

TRACE = False           # set by test harness for NTFF profiling
LAST_EXEC_NS = None
LAST_SCOPES = None


def _chunk_size(i):
    return min(CH, NPC - CH * i)


def _grid_windows(a, b):
    """Decompose [a, b) within [0,128) into legal matmul K-windows."""
    out = []
    while a < b:
        if a == 0 and b == 128:
            out.append((0, 128)); a = 128
        elif a % 64 == 0 and b - a >= 64:
            out.append((a, a + 64)); a += 64
        else:
            assert a % 32 == 0 and b - a >= 32, (a, b)
            out.append((a, a + 32)); a += 32
    return out


def _preprocess(edge_index):
    src = np.asarray(edge_index[0], dtype=np.int64)
    dst = np.asarray(edge_index[1], dtype=np.int64)
    E = src.shape[0]

    deg = np.bincount(dst, minlength=N).astype(np.float64) + 1.0
    dinv = 1.0 / np.sqrt(deg)
    ew = (dinv[src] * dinv[dst]) * (1.0 - ALPHA)
    selfw = ((dinv * dinv) * (1.0 - ALPHA)).astype(np.float32)

    core_d = dst // NPC
    dloc = dst % NPC
    chunk = dloc >> 7
    dcol = dloc & 127
    core_s = src // NPC
    sl = src % NPC
    p = sl & 127
    isrc = sl >> 7
    tab = (isrc >= SPLIT).astype(np.int64)
    nr = np.where(tab == 0, NR[0], NR[1])
    iloc = isrc - SPLIT * tab
    # row within the 4-core block window of table `tab`
    rowloc = (core_s % 4) * (128 * nr) + p * nr + iloc
    region = tab * 2 + core_s // 4

    # sort per (dst core, chunk, region, rowloc)
    order = np.lexsort((rowloc, region, chunk, core_d))
    core_so = core_d[order]
    chunk_o = chunk[order]; region_o = region[order]
    dcol_o = dcol[order]; rowloc_o = rowloc[order]
    w_o = ew[order].astype(np.float32)

    # counts per (core, chunk, region)
    key = (core_so * NCHK + chunk_o) * NREG + region_o
    nkeys = NCORE * NCHK * NREG
    starts = np.searchsorted(key, np.arange(nkeys), side="left")
    ends = np.searchsorted(key, np.arange(nkeys), side="right")
    cnts = (ends - starts).reshape(NCORE, NCHK, NREG)
    m_ir = cnts.max(axis=0)                      # [NCHK, NREG]
    alloc_ir = m_ir                              # unaligned; masking via w=0

    groups = []
    i = 0
    while i < NCHK:
        groups.append(list(range(i, min(i + GSIZE, NCHK))))
        i += GSIZE

    # --- slot layout ---------------------------------------------------
    # per (group, region): concat 32-aligned runs of the group's chunks,
    # pad to a 128 multiple; tiles of 128 slots; calls of <=16 tiles.
    tile_region = []          # region of each global tile
    tile_call = []            # call id of each global tile
    tile_tloc = []            # position within its call
    calls = []                # {t0, nt, region, group, idx_off16}
    run_slot = {}             # (i, r) -> global slot offset of run start
    seg_by_chunk = {i_: [] for i_ in range(NCHK)}   # list of (tile, a, b)
    calls_of_group = {}
    nslot = 0

    for gi, g in enumerate(groups):
        cg = []
        for r in range(NREG):
            gr_t0 = len(tile_region)          # first tile of this (g, r)
            base = nslot
            off = 0
            spans = []
            for i_ in g:
                run_slot[(i_, r)] = base + off
                spans.append((i_, off, off + alloc_ir[i_, r]))
                off += alloc_ir[i_, r]
            tot = ((off + 127) // 128) * 128
            ntile = tot // 128
            nslot = base + tot
            # tiles + calls (balanced split into <=MAX_CALL_TILES)
            ncall = max(1, -(-ntile // MAX_CALL_TILES))
            sizes = [ntile // ncall + (1 if k < ntile % ncall else 0)
                     for k in range(ncall)]
            t = gr_t0
            for snt in sizes:
                cid = len(calls)
                calls.append({"t0": t, "nt": snt, "region": r, "group": gi})
                for k in range(snt):
                    tile_region.append(r)
                    tile_call.append(cid)
                    tile_tloc.append(k)
                t += snt
                cg.append(cid)
            # matmul segments per chunk: one per overlapped tile
            for (i_, a, b) in spans:
                if a == b:
                    continue
                ta = a // 128
                while 128 * ta < b:
                    lo = max(a, 128 * ta) - 128 * ta
                    hi = min(b, 128 * (ta + 1)) - 128 * ta
                    seg_by_chunk[i_].append((gr_t0 + ta, lo, hi))
                    ta += 1
        calls_of_group[gi] = cg

    NT = len(tile_region)
    SLOTS = NT * CH
    assert nslot == SLOTS

    # --- per-core slot fills -------------------------------------------
    idx16 = np.zeros((NCORE, SLOTS), np.int16)
    dcol_f = np.zeros((NCORE, SLOTS), np.float32)
    w_f = np.zeros((NCORE, SLOTS), np.float32)
    for c in range(NCORE):
        for i_ in range(NCHK):
            for r in range(NREG):
                k = (c * NCHK + i_) * NREG + r
                s0, s1 = starts[k], ends[k]
                n = s1 - s0
                if n == 0:
                    continue
                b0 = run_slot[(i_, r)]
                idx16[c, b0:b0 + n] = rowloc_o[s0:s1].astype(np.int16)
                dcol_f[c, b0:b0 + n] = dcol_o[s0:s1].astype(np.float32)
                w_f[c, b0:b0 + n] = w_o[s0:s1]

    # idx wrapped into 16 partitions, replicated 8x across 128; per call
    idx_sb = np.zeros((NCORE, CH, SLOTS // 16), np.int16)
    off16 = 0
    for call in calls:
        call["idx_off16"] = off16
        nsl = call["nt"] * CH
        s0 = call["t0"] * CH
        for c in range(NCORE):
            seg = idx16[c, s0:s0 + nsl]
            idx_sb[c, :, off16:off16 + nsl // 16] = np.tile(
                seg.reshape(nsl // 16, 16).T, (8, 1))
        off16 += nsl // 16

    # per-SEGMENT dcol/w columns ([core, 128, NSEG]); rows outside the
    # segment get w=0 so the full-128 matmul is masked by data
    seg_list = []          # (tile, lo, hi) in global seg order
    seg_of_chunk = {}      # chunk -> list of seg ids
    segs_sorted = sorted(
        ((t, lo, hi, i_) for i_, sl_ in seg_by_chunk.items()
         for (t, lo, hi) in sl_), key=lambda x: (x[0], x[1]))
    for (t, lo, hi, i_) in segs_sorted:
        sid = len(seg_list)
        seg_list.append((t, lo, hi))
        seg_of_chunk.setdefault(i_, []).append(sid)
    NSEG = len(seg_list)
    dcol_sb = np.zeros((NCORE, CH, NSEG), np.float32)
    w_sb = np.zeros((NCORE, CH, NSEG), np.float32)
    for sid, (t, lo, hi) in enumerate(seg_list):
        s0 = t * CH
        dcol_sb[:, lo:hi, sid] = dcol_f[:, s0 + lo:s0 + hi]
        w_sb[:, lo:hi, sid] = w_f[:, s0 + lo:s0 + hi]
    seg_by_chunk = seg_of_chunk

    # self weights: [core, 128 (dcol), NCHK], scaled by (1-alpha)
    selfw_sb = np.zeros((NCORE, CH, NCHK), np.float32)
    for c in range(NCORE):
        sw = selfw[c * NPC:(c + 1) * NPC]
        swp = np.zeros(NCHK * CH, np.float32)
        swp[:NPC] = sw
        selfw_sb[c] = swp.reshape(NCHK, CH).T

    static = {"groups": groups, "calls": calls,
              "calls_of_group": calls_of_group,
              "seg_by_chunk": seg_by_chunk, "seg_list": seg_list,
              "tile_call": tile_call, "tile_tloc": tile_tloc,
              "NT": NT, "SLOTS": SLOTS, "NSEG": NSEG}
    per_core = {"idx_sb": idx_sb, "dcol_sb": dcol_sb, "w_sb": w_sb,
                "selfw_sb": selfw_sb}
    return static, per_core


def _install_ntff_hook():
    from concourse import bass_utils
    try:
        import antenv
        from trn_agent_boot.trn_boot import _ntff_profile_via_ctypes
    except Exception:
        return
    if "antenv.axon_hooks" in sys.modules:
        return
    mod = types.ModuleType("antenv.axon_hooks")
    state = {"hook": None}
    mod.set_axon_ntff_profile_hook = lambda h: state.__setitem__("hook", h)
    mod.get_axon_ntff_profile_hook = lambda: state["hook"]
    sys.modules["antenv.axon_hooks"] = mod
    antenv.axon_hooks = mod
    mod.set_axon_ntff_profile_hook(
        _ntff_profile_via_ctypes("/opt/axon/libaxon_pjrt.so"))
    bass_utils.upload_artifacts = lambda tmpdir: f"local:{tmpdir}"


def _build(static):
    import concourse.bass as bass
    import concourse.bacc as bacc
    import concourse.tile as tile
    import concourse.mybir as mybir
    from concourse.masks import make_identity

    f32 = mybir.dt.float32
    f16 = mybir.dt.float16
    i16 = mybir.dt.int16
    i32 = mybir.dt.int32
    AF = mybir.ActivationFunctionType
    OP = mybir.AluOpType
    AX = mybir.AxisListType

    groups = static["groups"]
    calls = static["calls"]
    calls_of_group = static["calls_of_group"]
    seg_by_chunk = static["seg_by_chunk"]
    tile_call = static["tile_call"]
    tile_tloc = static["tile_tloc"]
    seg_list = static["seg_list"]
    SLOTS = static["SLOTS"]
    NT = static["NT"]
    NSEG = static["NSEG"]
    # segments per group (for S pool ring sizing + build issue order)
    segs_of_group = {gi: [] for gi in range(len(groups))}
    tile_group = {}
    for gi in range(len(groups)):
        for cid in static["calls_of_group"][gi]:
            c = calls[cid]
            for t in range(c["t0"], c["t0"] + c["nt"]):
                tile_group[t] = gi
    for sid, (t, lo, hi) in enumerate(seg_list):
        segs_of_group[tile_group[t]].append(sid)

    nc = bacc.Bacc("TRN2", target_bir_lowering=False, debug=False,
                   num_devices=NCORE, num_swdge_queues=4)

    x_d = nc.dram_tensor("x_sh", [NPC, F_IN], f32, kind="ExternalInput").ap()
    W1_d = nc.dram_tensor("w1", [F_IN, HID], f32, kind="ExternalInput").ap()
    W2_d = nc.dram_tensor("w2", [HID, CLS], f32, kind="ExternalInput").ap()
    b1_d = nc.dram_tensor("b1c", [128, 2], f32, kind="ExternalInput").ap()
    b2_d = nc.dram_tensor("b2r", [128, CLS], f32, kind="ExternalInput").ap()
    idx_d = nc.dram_tensor("idxs", [128, SLOTS // 16], i16,
                           kind="ExternalInput").ap()
    dcol_d = nc.dram_tensor("dcols", [128, NSEG], f32,
                            kind="ExternalInput").ap()
    wv_d = nc.dram_tensor("wvals", [128, NSEG], f32,
                          kind="ExternalInput").ap()
    selfw_d = nc.dram_tensor("selfwf", [128, NCHK], f32,
                             kind="ExternalInput").ap()
    out_d = nc.dram_tensor("out", [NPC, CLS], f32, kind="ExternalOutput").ap()

    # staging + gathered tables, double-buffered by step parity
    hstage = [[nc.dram_tensor(f"hstage{t}_{pa}", [128, NR[t] * 128], f16).ap()
               for t in range(2)] for pa in range(2)]
    ztab = [[nc.dram_tensor(f"ztab{t}_{pa}", [NCORE * 128 * NR[t], 128], f16,
                            addr_space="Shared").ap()
             for t in range(2)] for pa in range(2)]

    def region_src(pa, r):
        t, b = r // 2, r % 2
        R4 = 4 * 128 * NR[t]
        return ztab[pa][t][R4 * b:R4 * (b + 1), :]

    with tile.TileContext(nc) as tc:
        with (
            tc.tile_pool(name="const", bufs=1) as cp,
            tc.tile_pool(name="resident", bufs=1) as rp,
            tc.tile_pool(name="mlp", bufs=3) as mp,
            tc.tile_pool(name="gb", bufs=18) as gp,
            tc.tile_pool(name="sb", bufs=192) as sp,
            tc.tile_pool(name="sm", bufs=4) as smp,
        ):
            # constants / residents
            idx_t = rp.tile([128, SLOTS // 16], i16)
            nc.sync.dma_start(idx_t[:], idx_d[:])
            dcol_t = rp.tile([128, NSEG], f32)
            nc.sync.dma_start(dcol_t[:], dcol_d[:])
            wv_t = rp.tile([128, NSEG], f32)
            nc.sync.dma_start(wv_t[:], wv_d[:])
            selfw_t = rp.tile([128, NCHK], f32)
            nc.sync.dma_start(selfw_t[:], selfw_d[:])
            W1_t = cp.tile([128, HID], f32)
            nc.sync.dma_start(W1_t[:], W1_d[:])
            W2a_t = cp.tile([128, CLS], f32)
            nc.sync.dma_start(W2a_t[:], W2_d[0:128, :])
            W2b_t = cp.tile([128, CLS], f32)
            nc.sync.dma_start(W2b_t[:], W2_d[128:256, :])
            b1_t = cp.tile([128, 2], f32)
            nc.sync.dma_start(b1_t[:], b1_d[:])
            b2_t = cp.tile([128, CLS], f32)
            nc.sync.dma_start(b2_t[:], b2_d[:])
            ident = cp.tile([128, 128], f32)
            make_identity(nc, ident[:])
            io32 = cp.tile([128, 128], i32)
            nc.gpsimd.iota(io32[:], pattern=[[1, 128]], base=0,
                           channel_multiplier=0)
            iota16 = cp.tile([128, 128], f16)
            nc.vector.tensor_scalar(out=iota16[:], in0=io32[:], scalar1=0.0,
                                    scalar2=None, op0=OP.add)

            ah_t = rp.tile([128, NCHK * 64], f16)
            nc.vector.memset(ah_t[:], 0.0)
            stgA = rp.tile([128, NCHK * 64], f16)
            nc.vector.memset(stgA[:], 0.0)
            stgB = rp.tile([128, NCHK * 64], f16)
            nc.vector.memset(stgB[:], 0.0)

            # ---- MLP: z0 = relu(x@W1+b1)@W2+b2 ----
            with tc.tile_pool(name="psmlp", bufs=2, space="PSUM") as pmp:
                for i in range(NCHK):
                    sz = _chunk_size(i)
                    xt = mp.tile([128, F_IN], f32, tag="xt")
                    nc.sync.dma_start(xt[0:sz, :], x_d[CH * i:CH * i + sz, :])
                    pxT = pmp.tile([128, 128], f32, tag="pmlp")
                    nc.tensor.transpose(pxT[:, 0:sz], xt[0:sz, :],
                                        ident[0:sz, 0:sz])
                    xT = mp.tile([128, 128], f32, tag="xT")
                    nc.scalar.activation(xT[:, 0:sz], pxT[:, 0:sz], AF.Copy)
                    relus = []
                    for h in range(2):
                        ph = pmp.tile([128, 128], f32, tag="pmlp")
                        nc.tensor.matmul(ph[:, 0:sz],
                                         lhsT=W1_t[:, 128 * h:128 * (h + 1)],
                                         rhs=xT[:, 0:sz], start=True,
                                         stop=True)
                        rh = mp.tile([128, 128], f32, tag=f"relu{h}")
                        nc.scalar.activation(rh[:, 0:sz], ph[:, 0:sz],
                                             AF.Relu, bias=b1_t[:, h:h + 1])
                        relus.append(rh)
                    pz = pmp.tile([128, 128], f32, tag="pmlp")
                    for h in range(2):
                        nc.tensor.matmul(pz[0:sz, 0:CLS],
                                         lhsT=relus[h][:, 0:sz],
                                         rhs=(W2a_t if h == 0 else W2b_t)[:],
                                         start=(h == 0), stop=(h == 1))
                    z0 = mp.tile([128, CLS], f32, tag="z0")
                    nc.vector.tensor_tensor(out=z0[0:sz, :],
                                            in0=pz[0:sz, 0:CLS],
                                            in1=b2_t[0:sz, :], op=OP.add)
                    nc.vector.tensor_copy(
                        out=stgA[0:sz, 64 * i:64 * i + CLS], in_=z0[0:sz, :])
                    nc.scalar.mul(ah_t[0:sz, 64 * i:64 * i + CLS],
                                  z0[0:sz, :], ALPHA)

            def stage_group(stg, pa, gi):
                g = groups[gi]
                t = 0 if g[0] < SPLIT else 1
                i0 = g[0] - (SPLIT if t else 0)
                i1 = g[-1] + 1 - (SPLIT if t else 0)
                hv = hstage[pa][t][:].rearrange("p (i f) -> p i f", f=128)
                sv = stg[:].rearrange("p (i f) -> p i f", f=64)
                nc.sync.dma_start(hv[:, i0:i1, 0:64],
                                  sv[:, g[0]:g[-1] + 1, :])

            def do_ag(pa, t):
                nc.gpsimd.collective_compute(
                    "AllGather", mybir.AluOpType.bypass,
                    replica_groups=[list(range(NCORE))],
                    ins=[hstage[pa][t][:].opt()],
                    outs=[ztab[pa][t][:].opt()])

            for gi in range(len(groups)):
                stage_group(stgA, 1, gi)
                if groups[gi][-1] == SPLIT - 1:
                    do_ag(1, 0)
            do_ag(1, 1)

            # ---- K propagation steps ----
            stg_prev, stg_new = stgA, stgB
            with tc.tile_pool(name="pschunk", bufs=8, space="PSUM") as psp:
                for k in range(1, K_ITERS + 1):
                    pa = k % 2          # parity of tables being READ
                    npa = (k + 1) % 2   # parity of tables being WRITTEN
                    for gi, grp in enumerate(groups):
                        gtile = {}
                        for cid in calls_of_group[gi]:
                            call = calls[cid]
                            nt = call["nt"]
                            r = call["region"]
                            g = gp.tile([128, MAX_CALL_TILES, 128], f16,
                                        tag="g")
                            nc.gpsimd.dma_gather(
                                g[:, 0:nt, :],
                                region_src(pa, r),
                                idx_t[:, call["idx_off16"]:
                                      call["idx_off16"] + nt * 8],
                                nt * 128, nt * 128, 128,
                                single_packet=SINGLE_PACKET,
                                queue_num=r,
                            )
                            gtile[cid] = g
                        # on-chip S build: one DVE op per segment
                        stile = {}
                        for sid in segs_of_group[gi]:
                            st = sp.tile([128, 128], f16, tag="st")
                            nc.vector.tensor_scalar(
                                out=st[:], in0=iota16[:],
                                scalar1=dcol_t[:, sid:sid + 1],
                                scalar2=wv_t[:, sid:sid + 1],
                                op0=OP.is_equal, op1=OP.mult)
                            stile[sid] = st
                        for i in grp:
                            sz = _chunk_size(i)
                            segs = seg_by_chunk[i]
                            ps = psp.tile([128, 64], f32, tag="ps")
                            for j, sid in enumerate(segs):
                                (t, lo, hi) = seg_list[sid]
                                cid = tile_call[t]
                                tloc = tile_tloc[t]
                                nc.tensor.matmul(
                                    ps[:, :],
                                    lhsT=stile[sid][:, :],
                                    rhs=gtile[cid][:, tloc, 0:64],
                                    start=(j == 0),
                                    stop=(j == len(segs) - 1))
                            # combine: z = ps + selfw*(1-a)*z_prev + a*h
                            t1 = smp.tile([128, 64], f16, tag="t1")
                            nc.vector.tensor_scalar(
                                out=t1[0:sz, :],
                                in0=stg_prev[0:sz, 64 * i:64 * (i + 1)],
                                scalar1=selfw_t[0:sz, i:i + 1],
                                scalar2=None, op0=OP.mult)
                            t2 = smp.tile([128, 64], f16, tag="t2")
                            nc.vector.tensor_tensor(
                                out=t2[0:sz, :], in0=t1[0:sz, :],
                                in1=ah_t[0:sz, 64 * i:64 * (i + 1)],
                                op=OP.add)
                            if k < K_ITERS:
                                nc.vector.tensor_tensor(
                                    out=stg_new[0:sz, 64 * i:64 * (i + 1)],
                                    in0=ps[0:sz, 0:64], in1=t2[0:sz, :],
                                    op=OP.add)
                            else:
                                z = smp.tile([128, CLS], f32, tag="z")
                                nc.vector.tensor_tensor(
                                    out=z[0:sz, :], in0=ps[0:sz, 0:CLS],
                                    in1=t2[0:sz, 0:CLS], op=OP.add)
                                m = smp.tile([128, 1], f32, tag="m")
                                nc.vector.tensor_reduce(
                                    m[0:sz, :], z[0:sz, :], axis=AX.X,
                                    op=OP.max)
                                nm = smp.tile([128, 1], f32, tag="nm")
                                nc.vector.tensor_scalar_mul(
                                    nm[0:sz, :], m[0:sz, :], -1.0)
                                e = smp.tile([128, CLS], f32, tag="e")
                                nc.scalar.activation(e[0:sz, :], z[0:sz, :],
                                                     AF.Exp,
                                                     bias=nm[0:sz, 0:1])
                                s = smp.tile([128, 1], f32, tag="s")
                                nc.vector.tensor_reduce(
                                    s[0:sz, :], e[0:sz, :], axis=AX.X,
                                    op=OP.add)
                                ls = smp.tile([128, 1], f32, tag="ls")
                                nc.scalar.activation(ls[0:sz, :], s[0:sz, :],
                                                     AF.Ln)
                                offs = smp.tile([128, 1], f32, tag="offs")
                                nc.vector.tensor_tensor(
                                    out=offs[0:sz, :], in0=m[0:sz, :],
                                    in1=ls[0:sz, :], op=OP.add)
                                res = smp.tile([128, CLS], f32, tag="res")
                                nc.vector.tensor_scalar(
                                    out=res[0:sz, :], in0=z[0:sz, :],
                                    scalar1=offs[0:sz, 0:1], scalar2=None,
                                    op0=OP.subtract)
                                nc.sync.dma_start(
                                    out_d[CH * i:CH * i + sz, :],
                                    res[0:sz, :])
                        if k < K_ITERS:
                            stage_group(stg_new, npa, gi)
                            if grp[-1] == SPLIT - 1:
                                do_ag(npa, 0)
                    if k < K_ITERS:
                        do_ag(npa, 1)
                        stg_prev, stg_new = stg_new, stg_prev

    nc.compile()
    return nc


def kernel(x, edge_index, W1, b1, W2, b2):
    global LAST_EXEC_NS, LAST_SCOPES
    from concourse import bass_utils

    x = np.asarray(x, np.float32)
    ei = np.asarray(edge_index)
    W1 = np.asarray(W1, np.float32)
    b1 = np.asarray(b1, np.float32)
    W2 = np.asarray(W2, np.float32)
    b2 = np.asarray(b2, np.float32)

    static, per_core = _preprocess(ei)
    nc = _build(static)

    b1c = np.stack([b1[0:128], b1[128:256]], axis=1).astype(np.float32)
    b1c = np.ascontiguousarray(b1c)
    b2r = np.ascontiguousarray(np.tile(b2[None, :], (128, 1)).astype(np.float32))

    in_maps = []
    for c in range(NCORE):
        in_maps.append({
            "x_sh": np.ascontiguousarray(x[c * NPC:(c + 1) * NPC]),
            "w1": W1, "w2": W2, "b1c": b1c, "b2r": b2r,
            "idxs": np.ascontiguousarray(per_core["idx_sb"][c]),
            "dcols": np.ascontiguousarray(per_core["dcol_sb"][c]),
            "wvals": np.ascontiguousarray(per_core["w_sb"][c]),
            "selfwf": np.ascontiguousarray(per_core["selfw_sb"][c]),
        })

    if TRACE:
        _install_ntff_hook()
    res = bass_utils.run_bass_kernel_spmd(
        nc, in_maps, core_ids=list(range(NCORE)), trace=TRACE)
    LAST_EXEC_NS = res.exec_time_ns
    LAST_SCOPES = res.per_core_scope_times

    out = np.concatenate([res.results[c]["out"] for c in range(NCORE)], axis=0)
    return out.astype(np.float32)


# revision 10
# speedup vs baseline: 1.9321x; 1.9321x over previous
"""APPNP GNN forward on 8 Trainium2 NeuronCores (Bass/Tile, SPMD).

Strategy (all 8 cores share one instruction stream; per-core data differs):
  - nodes sharded 12500/core; MLP data-parallel; z fp16 [128, NCHK*64] SBUF
  - z mirrored to HBM as FOUR quarter-tables (24/24/24/26 chunks), 256B
    rows, double-buffered by step parity; each quarter is AllGather'd as
    soon as its chunks are computed, so AGs overlap compute and the next
    step's gathers on earlier quarters
  - a quarter-table spans all 8 cores within int16 range, so gathers index
    the whole table (no block windows); region == quarter
  - edges partitioned by dst core; per (dst chunk, quarter) runs unaligned
    (SPMD max only); tiles of 128 slots span chunk boundaries
  - slots gathered via dma_gather (int16 row ids, 256B rows), queue=quarter
  - S matrices streamed from HBM as fp8e4m3 per-SEGMENT [128,128] tiles
    (one segment per chunk x tile overlap; rows outside the segment have
    w=0 so every matmul is a full-128 masked one)
  - aggregation via TensorE into per-chunk PSUM [128, 64]; alpha*h and the
    self-loop term are applied by DVE at copy-out (no init matmuls)
  - final iteration computes log_softmax on-chip from PSUM, fp32 out
"""
import sys, os, types

sys.path.insert(0, "/opt/trn_rl_repo")
import numpy as np

N = 100000
NCORE = 8
NPC = N // NCORE             # 12500
CH = 128
NCHK = (NPC + CH - 1) // CH  # 98
QB = (0, 24, 48, 72, 98)     # quarter chunk boundaries
NRQ = (24, 24, 24, 26)       # chunks per quarter
GSIZE = 8
ALPHA = 0.1
MAX_CALL_TILES = 12
K_ITERS = 10
F_IN = 128
HID = 256
CLS = 47
NREG = 4                     # regions == quarters
AG_AFTER_GROUP = (2, 5, 8, 12)
SINGLE_PACKET = False
S_FP8 = True

TRACE = False           # set by test harness for NTFF profiling
LAST_EXEC_NS = None
LAST_SCOPES = None


def _chunk_size(i):
    return min(CH, NPC - CH * i)


def _preprocess(edge_index):
    import ml_dtypes

    src = np.asarray(edge_index[0], dtype=np.int64)
    dst = np.asarray(edge_index[1], dtype=np.int64)

    deg = np.bincount(dst, minlength=N).astype(np.float64) + 1.0
    dinv = 1.0 / np.sqrt(deg)
    ew = (dinv[src] * dinv[dst]) * (1.0 - ALPHA)
    selfw = ((dinv * dinv) * (1.0 - ALPHA)).astype(np.float32)

    core_d = dst // NPC
    dloc = dst % NPC
    chunk = dloc >> 7
    dcol = dloc & 127
    core_s = src // NPC
    sl = src % NPC
    p = sl & 127
    isrc = sl >> 7
    qb = np.asarray(QB)
    region = np.searchsorted(qb, isrc, side="right") - 1
    nrq = np.asarray(NRQ)[region]
    rowloc = core_s * (128 * nrq) + p * nrq + (isrc - qb[region])

    order = np.lexsort((rowloc, region, chunk, core_d))
    core_so = core_d[order]
    chunk_o = chunk[order]; region_o = region[order]
    dcol_o = dcol[order]; rowloc_o = rowloc[order]
    w_o = ew[order].astype(np.float32)

    key = (core_so * NCHK + chunk_o) * NREG + region_o
    nkeys = NCORE * NCHK * NREG
    starts = np.searchsorted(key, np.arange(nkeys), side="left")
    ends = np.searchsorted(key, np.arange(nkeys), side="right")
    cnts = (ends - starts).reshape(NCORE, NCHK, NREG)
    alloc_ir = cnts.max(axis=0)                  # [NCHK, NREG], unaligned

    groups = []
    i = 0
    while i < NCHK:
        groups.append(list(range(i, min(i + GSIZE, NCHK))))
        i += GSIZE

    # --- slot layout: per (group, region) concat runs, pad to 128 --------
    tile_call = []
    tile_tloc = []
    calls = []
    run_slot = {}
    raw_segs = []             # (tile, lo, hi, chunk) in (tile, lo) order
    calls_of_group = {}
    nslot = 0

    for gi, g in enumerate(groups):
        cg = []
        for r in range(NREG):
            gr_t0 = len(tile_call)
            base = nslot
            off = 0
            spans = []
            for i_ in g:
                run_slot[(i_, r)] = base + off
                spans.append((i_, off, off + alloc_ir[i_, r]))
                off += alloc_ir[i_, r]
            tot = ((off + 127) // 128) * 128
            ntile = tot // 128
            nslot = base + tot
            ncall = max(1, -(-ntile // MAX_CALL_TILES))
            sizes = [ntile // ncall + (1 if k < ntile % ncall else 0)
                     for k in range(ncall)]
            t = gr_t0
            for snt in sizes:
                cid = len(calls)
                calls.append({"t0": t, "nt": snt, "region": r, "group": gi})
                for k in range(snt):
                    tile_call.append(cid)
                    tile_tloc.append(k)
                t += snt
                cg.append(cid)
            for (i_, a, b) in spans:
                if a == b:
                    continue
                ta = a // 128
                while 128 * ta < b:
                    lo = max(a, 128 * ta) - 128 * ta
                    hi = min(b, 128 * (ta + 1)) - 128 * ta
                    raw_segs.append((gr_t0 + ta, lo, hi, i_))
                    ta += 1
        calls_of_group[gi] = cg

    NT = len(tile_call)
    SLOTS = NT * CH
    assert nslot == SLOTS

    # segments: global order by (tile, lo); contiguous per call
    raw_segs.sort(key=lambda x: (x[0], x[1]))
    seg_list = [(t, lo, hi) for (t, lo, hi, _) in raw_segs]
    seg_by_chunk = {}
    for sid, (t, lo, hi, i_) in enumerate(raw_segs):
        seg_by_chunk.setdefault(i_, []).append(sid)
    NSEG = len(seg_list)
    seg_lo = {}
    seg_cnt = {}
    for sid, (t, lo, hi) in enumerate(seg_list):
        cid = tile_call[t]
        seg_lo.setdefault(cid, sid)
        seg_cnt[cid] = seg_cnt.get(cid, 0) + 1
    for cid, call in enumerate(calls):
        call["s0"] = seg_lo[cid]
        call["ns"] = seg_cnt[cid]
    MAXSEGC = max(c["ns"] for c in calls)

    # --- per-core slot fills -------------------------------------------
    idx16 = np.zeros((NCORE, SLOTS), np.int16)
    dcol_f = np.zeros((NCORE, SLOTS), np.int64)
    w_f = np.zeros((NCORE, SLOTS), np.float32)
    for c in range(NCORE):
        for i_ in range(NCHK):
            for r in range(NREG):
                k = (c * NCHK + i_) * NREG + r
                s0, s1 = starts[k], ends[k]
                n = s1 - s0
                if n == 0:
                    continue
                b0 = run_slot[(i_, r)]
                idx16[c, b0:b0 + n] = rowloc_o[s0:s1].astype(np.int16)
                dcol_f[c, b0:b0 + n] = dcol_o[s0:s1]
                w_f[c, b0:b0 + n] = w_o[s0:s1]

    # idx wrapped into 16 partitions, replicated 8x across 128; per call
    idx_sb = np.zeros((NCORE, CH, SLOTS // 16), np.int16)
    off16 = 0
    for call in calls:
        call["idx_off16"] = off16
        nsl = call["nt"] * CH
        s0 = call["t0"] * CH
        for c in range(NCORE):
            seg = idx16[c, s0:s0 + nsl]
            idx_sb[c, :, off16:off16 + nsl // 16] = np.tile(
                seg.reshape(nsl // 16, 16).T, (8, 1))
        off16 += nsl // 16

    # --- dense per-segment S tiles, fp8e4m3: [core, 128, NSEG*128] ------
    sdt = ml_dtypes.float8_e4m3 if S_FP8 else np.float16
    sdata = np.zeros((NCORE, CH, NSEG, CH), sdt)
    wq = w_f.astype(sdt)
    for sid, (t, lo, hi) in enumerate(seg_list):
        s0 = t * CH
        rows = np.arange(lo, hi)
        for c in range(NCORE):
            sdata[c, rows, sid, dcol_f[c, s0 + lo:s0 + hi]] = \
                wq[c, s0 + lo:s0 + hi]
    sdata = np.ascontiguousarray(sdata.reshape(NCORE, CH, NSEG * CH))

    selfw_sb = np.zeros((NCORE, CH, NCHK), np.float32)
    for c in range(NCORE):
        sw = selfw[c * NPC:(c + 1) * NPC]
        swp = np.zeros(NCHK * CH, np.float32)
        swp[:NPC] = sw
        selfw_sb[c] = swp.reshape(NCHK, CH).T

    static = {"groups": groups, "calls": calls,
              "calls_of_group": calls_of_group,
              "seg_by_chunk": seg_by_chunk, "seg_list": seg_list,
              "tile_call": tile_call, "tile_tloc": tile_tloc,
              "NT": NT, "SLOTS": SLOTS, "NSEG": NSEG, "MAXSEGC": MAXSEGC}
    per_core = {"idx_sb": idx_sb, "sdata": sdata, "selfw_sb": selfw_sb}
    return static, per_core


def _install_ntff_hook():
    from concourse import bass_utils
    try:
        import antenv
        from trn_agent_boot.trn_boot import _ntff_profile_via_ctypes
    except Exception:
        return
    if "antenv.axon_hooks" in sys.modules:
        return
    mod = types.ModuleType("antenv.axon_hooks")
    state = {"hook": None}
    mod.set_axon_ntff_profile_hook = lambda h: state.__setitem__("hook", h)
    mod.get_axon_ntff_profile_hook = lambda: state["hook"]
    sys.modules["antenv.axon_hooks"] = mod
    antenv.axon_hooks = mod
    mod.set_axon_ntff_profile_hook(
        _ntff_profile_via_ctypes("/opt/axon/libaxon_pjrt.so"))
    bass_utils.upload_artifacts = lambda tmpdir: f"local:{tmpdir}"


def _build(static):
    import concourse.bass as bass
    import concourse.bacc as bacc
    import concourse.tile as tile
    import concourse.mybir as mybir
    from concourse.masks import make_identity

    f32 = mybir.dt.float32
    f16 = mybir.dt.float16
    f8 = mybir.dt.float8e4 if S_FP8 else mybir.dt.float16
    i16 = mybir.dt.int16
    AF = mybir.ActivationFunctionType
    OP = mybir.AluOpType
    AX = mybir.AxisListType

    groups = static["groups"]
    calls = static["calls"]
    calls_of_group = static["calls_of_group"]
    seg_by_chunk = static["seg_by_chunk"]
    seg_list = static["seg_list"]
    tile_call = static["tile_call"]
    tile_tloc = static["tile_tloc"]
    SLOTS = static["SLOTS"]
    NSEG = static["NSEG"]
    MAXSEGC = static["MAXSEGC"]

    nc = bacc.Bacc("TRN2", target_bir_lowering=False, debug=False,
                   num_devices=NCORE, num_swdge_queues=4)

    x_d = nc.dram_tensor("x_sh", [NPC, F_IN], f32, kind="ExternalInput").ap()
    W1_d = nc.dram_tensor("w1", [F_IN, HID], f32, kind="ExternalInput").ap()
    W2_d = nc.dram_tensor("w2", [HID, CLS], f32, kind="ExternalInput").ap()
    b1_d = nc.dram_tensor("b1c", [128, 2], f32, kind="ExternalInput").ap()
    b2_d = nc.dram_tensor("b2r", [128, CLS], f32, kind="ExternalInput").ap()
    idx_d = nc.dram_tensor("idxs", [128, SLOTS // 16], i16,
                           kind="ExternalInput").ap()
    sdata_d = nc.dram_tensor("sdata", [128, NSEG * 128], f8,
                             kind="ExternalInput").ap()
    selfw_d = nc.dram_tensor("selfwf", [128, NCHK], f32,
                             kind="ExternalInput").ap()
    out_d = nc.dram_tensor("out", [NPC, CLS], f32, kind="ExternalOutput").ap()

    hstage = [[nc.dram_tensor(f"hstage{q}_{pa}", [128, NRQ[q] * 128], f16).ap()
               for q in range(4)] for pa in range(2)]
    ztab = [[nc.dram_tensor(f"ztab{q}_{pa}", [NCORE * 128 * NRQ[q], 128],
                            f16, addr_space="Shared").ap()
             for q in range(4)] for pa in range(2)]

    with tile.TileContext(nc) as tc:
        with (
            tc.tile_pool(name="const", bufs=1) as cp,
            tc.tile_pool(name="resident", bufs=1) as rp,
            tc.tile_pool(name="mlp", bufs=3) as mp,
            tc.tile_pool(name="gb", bufs=18) as gp,
            tc.tile_pool(name="sb", bufs=18) as sp,
            tc.tile_pool(name="sm", bufs=4) as smp,
        ):
            # constants / residents
            idx_t = rp.tile([128, SLOTS // 16], i16)
            nc.sync.dma_start(idx_t[:], idx_d[:])
            selfw_t = rp.tile([128, NCHK], f32)
            nc.sync.dma_start(selfw_t[:], selfw_d[:])
            W1_t = cp.tile([128, HID], f32)
            nc.sync.dma_start(W1_t[:], W1_d[:])
            W2a_t = cp.tile([128, CLS], f32)
            nc.sync.dma_start(W2a_t[:], W2_d[0:128, :])
            W2b_t = cp.tile([128, CLS], f32)
            nc.sync.dma_start(W2b_t[:], W2_d[128:256, :])
            b1_t = cp.tile([128, 2], f32)
            nc.sync.dma_start(b1_t[:], b1_d[:])
            b2_t = cp.tile([128, CLS], f32)
            nc.sync.dma_start(b2_t[:], b2_d[:])
            ident = cp.tile([128, 128], f32)
            make_identity(nc, ident[:])

            ah_t = rp.tile([128, NCHK * 64], f16)
            nc.vector.memset(ah_t[:], 0.0)
            stgA = rp.tile([128, NCHK * 64], f16)
            nc.vector.memset(stgA[:], 0.0)
            stgB = rp.tile([128, NCHK * 64], f16)
            nc.vector.memset(stgB[:], 0.0)

            # ---- MLP: z0 = relu(x@W1+b1)@W2+b2 ----
            with tc.tile_pool(name="psmlp", bufs=2, space="PSUM") as pmp:
                for i in range(NCHK):
                    sz = _chunk_size(i)
                    xt = mp.tile([128, F_IN], f32, tag="xt")
                    nc.sync.dma_start(xt[0:sz, :], x_d[CH * i:CH * i + sz, :])
                    pxT = pmp.tile([128, 128], f32, tag="pmlp")
                    nc.tensor.transpose(pxT[:, 0:sz], xt[0:sz, :],
                                        ident[0:sz, 0:sz])
                    xT = mp.tile([128, 128], f32, tag="xT")
                    nc.scalar.activation(xT[:, 0:sz], pxT[:, 0:sz], AF.Copy)
                    relus = []
                    for h in range(2):
                        ph = pmp.tile([128, 128], f32, tag="pmlp")
                        nc.tensor.matmul(ph[:, 0:sz],
                                         lhsT=W1_t[:, 128 * h:128 * (h + 1)],
                                         rhs=xT[:, 0:sz], start=True,
                                         stop=True)
                        rh = mp.tile([128, 128], f32, tag=f"relu{h}")
                        nc.scalar.activation(rh[:, 0:sz], ph[:, 0:sz],
                                             AF.Relu, bias=b1_t[:, h:h + 1])
                        relus.append(rh)
                    pz = pmp.tile([128, 128], f32, tag="pmlp")
                    for h in range(2):
                        nc.tensor.matmul(pz[0:sz, 0:CLS],
                                         lhsT=relus[h][:, 0:sz],
                                         rhs=(W2a_t if h == 0 else W2b_t)[:],
                                         start=(h == 0), stop=(h == 1))
                    z0 = mp.tile([128, CLS], f32, tag="z0")
                    nc.vector.tensor_tensor(out=z0[0:sz, :],
                                            in0=pz[0:sz, 0:CLS],
                                            in1=b2_t[0:sz, :], op=OP.add)
                    nc.vector.tensor_copy(
                        out=stgA[0:sz, 64 * i:64 * i + CLS], in_=z0[0:sz, :])
                    nc.scalar.mul(ah_t[0:sz, 64 * i:64 * i + CLS],
                                  z0[0:sz, :], ALPHA)

            def stage_group(stg, pa, gi):
                g = groups[gi]
                q = 0
                while g[0] >= QB[q + 1]:
                    q += 1
                i0 = g[0] - QB[q]
                i1 = g[-1] + 1 - QB[q]
                hv = hstage[pa][q][:].rearrange("p (i f) -> p i f", f=128)
                sv = stg[:].rearrange("p (i f) -> p i f", f=64)
                nc.sync.dma_start(hv[:, i0:i1, 0:64],
                                  sv[:, g[0]:g[-1] + 1, :])

            def do_ag(pa, q):
                nc.gpsimd.collective_compute(
                    "AllGather", mybir.AluOpType.bypass,
                    replica_groups=[list(range(NCORE))],
                    ins=[hstage[pa][q][:].opt()],
                    outs=[ztab[pa][q][:].opt()])

            for gi in range(len(groups)):
                stage_group(stgA, 1, gi)
                if gi in AG_AFTER_GROUP:
                    do_ag(1, AG_AFTER_GROUP.index(gi))

            # ---- K propagation steps ----
            stg_prev, stg_new = stgA, stgB
            with tc.tile_pool(name="pschunk", bufs=8, space="PSUM") as psp:
                for k in range(1, K_ITERS + 1):
                    pa = k % 2
                    npa = (k + 1) % 2
                    for gi, grp in enumerate(groups):
                        gtile = {}
                        stile = {}
                        for cid in calls_of_group[gi]:
                            call = calls[cid]
                            nt = call["nt"]
                            r = call["region"]
                            g = gp.tile([128, MAX_CALL_TILES, 128], f16,
                                        tag="g")
                            nc.gpsimd.dma_gather(
                                g[:, 0:nt, :],
                                ztab[pa][r][:],
                                idx_t[:, call["idx_off16"]:
                                      call["idx_off16"] + nt * 8],
                                nt * 128, nt * 128, 128,
                                single_packet=SINGLE_PACKET,
                                queue_num=r,
                            )
                            gtile[cid] = g
                            st = sp.tile([128, MAXSEGC * 128], f8, tag="st")
                            ns = call["ns"]
                            s0 = call["s0"]
                            nc.sync.dma_start(
                                st[:, 0:ns * 128],
                                sdata_d[:, s0 * 128:(s0 + ns) * 128])
                            stile[cid] = st
                        for i in grp:
                            sz = _chunk_size(i)
                            segs = seg_by_chunk[i]
                            ps = psp.tile([128, 64], f32, tag="ps")
                            for j, sid in enumerate(segs):
                                (t, lo, hi) = seg_list[sid]
                                cid = tile_call[t]
                                tloc = tile_tloc[t]
                                sj = sid - calls[cid]["s0"]
                                nc.tensor.matmul(
                                    ps[:, :],
                                    lhsT=stile[cid][:, 128 * sj:
                                                    128 * (sj + 1)],
                                    rhs=gtile[cid][:, tloc, 0:64],
                                    start=(j == 0),
                                    stop=(j == len(segs) - 1))
                            # combine: z = ps + selfw*(1-a)*z_prev + a*h
                            t1 = smp.tile([128, 64], f16, tag="t1")
                            nc.vector.tensor_scalar(
                                out=t1[0:sz, :],
                                in0=stg_prev[0:sz, 64 * i:64 * (i + 1)],
                                scalar1=selfw_t[0:sz, i:i + 1],
                                scalar2=None, op0=OP.mult)
                            t2 = smp.tile([128, 64], f16, tag="t2")
                            nc.vector.tensor_tensor(
                                out=t2[0:sz, :], in0=t1[0:sz, :],
                                in1=ah_t[0:sz, 64 * i:64 * (i + 1)],
                                op=OP.add)
                            if k < K_ITERS:
                                nc.vector.tensor_tensor(
                                    out=stg_new[0:sz, 64 * i:64 * (i + 1)],
                                    in0=ps[0:sz, 0:64], in1=t2[0:sz, :],
                                    op=OP.add)
                            else:
                                z = smp.tile([128, CLS], f32, tag="z")
                                nc.vector.tensor_tensor(
                                    out=z[0:sz, :], in0=ps[0:sz, 0:CLS],
                                    in1=t2[0:sz, 0:CLS], op=OP.add)
                                m = smp.tile([128, 1], f32, tag="m")
                                nc.vector.tensor_reduce(
                                    m[0:sz, :], z[0:sz, :], axis=AX.X,
                                    op=OP.max)
                                nm = smp.tile([128, 1], f32, tag="nm")
                                nc.vector.tensor_scalar_mul(
                                    nm[0:sz, :], m[0:sz, :], -1.0)
                                e = smp.tile([128, CLS], f32, tag="e")
                                nc.scalar.activation(e[0:sz, :], z[0:sz, :],
                                                     AF.Exp,
                                                     bias=nm[0:sz, 0:1])
                                s = smp.tile([128, 1], f32, tag="s")
                                nc.vector.tensor_reduce(
                                    s[0:sz, :], e[0:sz, :], axis=AX.X,
                                    op=OP.add)
                                ls = smp.tile([128, 1], f32, tag="ls")
                                nc.scalar.activation(ls[0:sz, :], s[0:sz, :],
                                                     AF.Ln)
                                offs = smp.tile([128, 1], f32, tag="offs")
                                nc.vector.tensor_tensor(
                                    out=offs[0:sz, :], in0=m[0:sz, :],
                                    in1=ls[0:sz, :], op=OP.add)
                                res = smp.tile([128, CLS], f32, tag="res")
                                nc.vector.tensor_scalar(
                                    out=res[0:sz, :], in0=z[0:sz, :],
                                    scalar1=offs[0:sz, 0:1], scalar2=None,
                                    op0=OP.subtract)
                                nc.sync.dma_start(
                                    out_d[CH * i:CH * i + sz, :],
                                    res[0:sz, :])
                        if k < K_ITERS:
                            stage_group(stg_new, npa, gi)
                            if gi in AG_AFTER_GROUP:
                                do_ag(npa, AG_AFTER_GROUP.index(gi))
                    if k < K_ITERS:
                        stg_prev, stg_new = stg_new, stg_prev

    nc.compile()
    return nc


def kernel(x, edge_index, W1, b1, W2, b2):
    global LAST_EXEC_NS, LAST_SCOPES
    from concourse import bass_utils

    x = np.asarray(x, np.float32)
    ei = np.asarray(edge_index)
    W1 = np.asarray(W1, np.float32)
    b1 = np.asarray(b1, np.float32)
    W2 = np.asarray(W2, np.float32)
    b2 = np.asarray(b2, np.float32)

    static, per_core = _preprocess(ei)
    nc = _build(static)

    b1c = np.stack([b1[0:128], b1[128:256]], axis=1).astype(np.float32)
    b1c = np.ascontiguousarray(b1c)
    b2r = np.ascontiguousarray(np.tile(b2[None, :], (128, 1)).astype(np.float32))

    in_maps = []
    for c in range(NCORE):
        in_maps.append({
            "x_sh": np.ascontiguousarray(x[c * NPC:(c + 1) * NPC]),
            "w1": W1, "w2": W2, "b1c": b1c, "b2r": b2r,
            "idxs": np.ascontiguousarray(per_core["idx_sb"][c]),
            "sdata": np.ascontiguousarray(per_core["sdata"][c]),
            "selfwf": np.ascontiguousarray(per_core["selfw_sb"][c]),
        })

    if TRACE:
        _install_ntff_hook()
    res = bass_utils.run_bass_kernel_spmd(
        nc, in_maps, core_ids=list(range(NCORE)), trace=TRACE)
    LAST_EXEC_NS = res.exec_time_ns
    LAST_SCOPES = res.per_core_scope_times

    out = np.concatenate([res.results[c]["out"] for c in range(NCORE)], axis=0)
    return out.astype(np.float32)


# revision 12
# speedup vs baseline: 1.9527x; 1.0107x over previous
"""APPNP GNN forward on 8 Trainium2 NeuronCores (Bass/Tile, SPMD).

Strategy (all 8 cores share one instruction stream; per-core data differs):
  - nodes sharded 12500/core; MLP data-parallel; z fp16 [128, NCHK*64] SBUF
  - z mirrored to HBM as FOUR quarter-tables (24/24/24/26 chunks), 256B
    rows, double-buffered by step parity; each quarter is AllGather'd as
    soon as its chunks are computed, so AGs overlap compute and the next
    step's gathers on earlier quarters
  - a quarter-table spans all 8 cores within int16 range, so gathers index
    the whole table (no block windows); region == quarter
  - edges partitioned by dst core; per (dst chunk, quarter) runs unaligned
    (SPMD max only); tiles of 128 slots span chunk boundaries
  - slots gathered via dma_gather (int16 row ids, 256B rows), queue=quarter
  - S matrices streamed from HBM as fp8e4m3 per-SEGMENT [128,128] tiles
    (one segment per chunk x tile overlap; rows outside the segment have
    w=0 so every matmul is a full-128 masked one)
  - aggregation via TensorE into per-chunk PSUM [128, 64]; alpha*h and the
    self-loop term are applied by DVE at copy-out (no init matmuls)
  - final iteration computes log_softmax on-chip from PSUM, fp32 out
"""
import sys, os, types

sys.path.insert(0, "/opt/trn_rl_repo")
import numpy as np

N = 100000
NCORE = 8
NPC = N // NCORE             # 12500
CH = 128
NCHK = (NPC + CH - 1) // CH  # 98
QB = (0, 24, 48, 72, 98)     # quarter chunk boundaries
NRQ = (24, 24, 24, 26)       # chunks per quarter
GSIZE = 8
ALPHA = 0.1
MAX_CALL_TILES = 12
K_ITERS = 10
F_IN = 128
HID = 256
CLS = 47
NREG = 4                     # regions == quarters
AG_AFTER_GROUP = (2, 5, 8, 12)
SINGLE_PACKET = False
S_FP8 = True

TRACE = False           # set by test harness for NTFF profiling
LAST_EXEC_NS = None
LAST_SCOPES = None


def _chunk_size(i):
    return min(CH, NPC - CH * i)


def _preprocess(edge_index):
    import ml_dtypes

    src = np.asarray(edge_index[0], dtype=np.int64)
    dst = np.asarray(edge_index[1], dtype=np.int64)

    deg = np.bincount(dst, minlength=N).astype(np.float64) + 1.0
    dinv = 1.0 / np.sqrt(deg)
    ew = (dinv[src] * dinv[dst]) * (1.0 - ALPHA)
    selfw = ((dinv * dinv) * (1.0 - ALPHA)).astype(np.float32)

    core_d = dst // NPC
    dloc = dst % NPC
    chunk = dloc >> 7
    dcol = dloc & 127
    core_s = src // NPC
    sl = src % NPC
    p = sl & 127
    isrc = sl >> 7
    qb = np.asarray(QB)
    region = np.searchsorted(qb, isrc, side="right") - 1
    nrq = np.asarray(NRQ)[region]
    rowloc = core_s * (128 * nrq) + p * nrq + (isrc - qb[region])

    order = np.lexsort((rowloc, region, chunk, core_d))
    core_so = core_d[order]
    chunk_o = chunk[order]; region_o = region[order]
    dcol_o = dcol[order]; rowloc_o = rowloc[order]
    w_o = ew[order].astype(np.float32)

    key = (core_so * NCHK + chunk_o) * NREG + region_o
    nkeys = NCORE * NCHK * NREG
    starts = np.searchsorted(key, np.arange(nkeys), side="left")
    ends = np.searchsorted(key, np.arange(nkeys), side="right")
    cnts = (ends - starts).reshape(NCORE, NCHK, NREG)
    alloc_ir = cnts.max(axis=0)                  # [NCHK, NREG], unaligned

    groups = []
    i = 0
    while i < NCHK:
        groups.append(list(range(i, min(i + GSIZE, NCHK))))
        i += GSIZE

    # --- slot layout: per (group, region) concat runs, pad to 128 --------
    tile_call = []
    tile_tloc = []
    calls = []
    run_slot = {}
    raw_segs = []             # (tile, lo, hi, chunk) in (tile, lo) order
    calls_of_group = {}
    nslot = 0

    for gi, g in enumerate(groups):
        cg = []
        for r in range(NREG):
            gr_t0 = len(tile_call)
            base = nslot
            off = 0
            spans = []
            for i_ in g:
                run_slot[(i_, r)] = base + off
                spans.append((i_, off, off + alloc_ir[i_, r]))
                off += alloc_ir[i_, r]
            tot = ((off + 127) // 128) * 128
            ntile = tot // 128
            nslot = base + tot
            ncall = max(1, -(-ntile // MAX_CALL_TILES))
            sizes = [ntile // ncall + (1 if k < ntile % ncall else 0)
                     for k in range(ncall)]
            t = gr_t0
            for snt in sizes:
                cid = len(calls)
                calls.append({"t0": t, "nt": snt, "region": r, "group": gi})
                for k in range(snt):
                    tile_call.append(cid)
                    tile_tloc.append(k)
                t += snt
                cg.append(cid)
            for (i_, a, b) in spans:
                if a == b:
                    continue
                ta = a // 128
                while 128 * ta < b:
                    lo = max(a, 128 * ta) - 128 * ta
                    hi = min(b, 128 * (ta + 1)) - 128 * ta
                    raw_segs.append((gr_t0 + ta, lo, hi, i_))
                    ta += 1
        calls_of_group[gi] = cg

    NT = len(tile_call)
    SLOTS = NT * CH
    assert nslot == SLOTS

    # segments: global order by (tile, lo); contiguous per call
    raw_segs.sort(key=lambda x: (x[0], x[1]))
    seg_list = [(t, lo, hi) for (t, lo, hi, _) in raw_segs]
    seg_by_chunk = {}
    for sid, (t, lo, hi, i_) in enumerate(raw_segs):
        seg_by_chunk.setdefault(i_, []).append(sid)
    NSEG = len(seg_list)
    seg_lo = {}
    seg_cnt = {}
    for sid, (t, lo, hi) in enumerate(seg_list):
        cid = tile_call[t]
        seg_lo.setdefault(cid, sid)
        seg_cnt[cid] = seg_cnt.get(cid, 0) + 1
    for cid, call in enumerate(calls):
        call["s0"] = seg_lo[cid]
        call["ns"] = seg_cnt[cid]
    MAXSEGC = max(c["ns"] for c in calls)

    # --- per-core slot fills -------------------------------------------
    idx16 = np.zeros((NCORE, SLOTS), np.int16)
    dcol_f = np.zeros((NCORE, SLOTS), np.int64)
    w_f = np.zeros((NCORE, SLOTS), np.float32)
    for c in range(NCORE):
        for i_ in range(NCHK):
            for r in range(NREG):
                k = (c * NCHK + i_) * NREG + r
                s0, s1 = starts[k], ends[k]
                n = s1 - s0
                if n == 0:
                    continue
                b0 = run_slot[(i_, r)]
                idx16[c, b0:b0 + n] = rowloc_o[s0:s1].astype(np.int16)
                dcol_f[c, b0:b0 + n] = dcol_o[s0:s1]
                w_f[c, b0:b0 + n] = w_o[s0:s1]

    # idx wrapped into 16 partitions, replicated 8x across 128; per call
    idx_sb = np.zeros((NCORE, CH, SLOTS // 16), np.int16)
    off16 = 0
    for call in calls:
        call["idx_off16"] = off16
        nsl = call["nt"] * CH
        s0 = call["t0"] * CH
        for c in range(NCORE):
            seg = idx16[c, s0:s0 + nsl]
            idx_sb[c, :, off16:off16 + nsl // 16] = np.tile(
                seg.reshape(nsl // 16, 16).T, (8, 1))
        off16 += nsl // 16

    # --- dense per-segment S tiles, fp8e4m3: [core, 128, NSEG*128] ------
    sdt = ml_dtypes.float8_e4m3 if S_FP8 else np.float16
    sdata = np.zeros((NCORE, CH, NSEG, CH), sdt)
    wq = w_f.astype(sdt)
    for sid, (t, lo, hi) in enumerate(seg_list):
        s0 = t * CH
        rows = np.arange(lo, hi)
        for c in range(NCORE):
            sdata[c, rows, sid, dcol_f[c, s0 + lo:s0 + hi]] = \
                wq[c, s0 + lo:s0 + hi]
    sdata = np.ascontiguousarray(sdata.reshape(NCORE, CH, NSEG * CH))

    selfw_sb = np.zeros((NCORE, CH, NCHK), np.float32)
    for c in range(NCORE):
        sw = selfw[c * NPC:(c + 1) * NPC]
        swp = np.zeros(NCHK * CH, np.float32)
        swp[:NPC] = sw
        selfw_sb[c] = swp.reshape(NCHK, CH).T

    static = {"groups": groups, "calls": calls,
              "calls_of_group": calls_of_group,
              "seg_by_chunk": seg_by_chunk, "seg_list": seg_list,
              "tile_call": tile_call, "tile_tloc": tile_tloc,
              "NT": NT, "SLOTS": SLOTS, "NSEG": NSEG, "MAXSEGC": MAXSEGC}
    per_core = {"idx_sb": idx_sb, "sdata": sdata, "selfw_sb": selfw_sb}
    return static, per_core


def _install_ntff_hook():
    from concourse import bass_utils
    try:
        import antenv
        from trn_agent_boot.trn_boot import _ntff_profile_via_ctypes
    except Exception:
        return
    if "antenv.axon_hooks" in sys.modules:
        return
    mod = types.ModuleType("antenv.axon_hooks")
    state = {"hook": None}
    mod.set_axon_ntff_profile_hook = lambda h: state.__setitem__("hook", h)
    mod.get_axon_ntff_profile_hook = lambda: state["hook"]
    sys.modules["antenv.axon_hooks"] = mod
    antenv.axon_hooks = mod
    mod.set_axon_ntff_profile_hook(
        _ntff_profile_via_ctypes("/opt/axon/libaxon_pjrt.so"))
    bass_utils.upload_artifacts = lambda tmpdir: f"local:{tmpdir}"


def _build(static):
    import concourse.bass as bass
    import concourse.bacc as bacc
    import concourse.tile as tile
    import concourse.mybir as mybir
    from concourse.masks import make_identity

    f32 = mybir.dt.float32
    f16 = mybir.dt.float16
    f8 = mybir.dt.float8e4 if S_FP8 else mybir.dt.float16
    i16 = mybir.dt.int16
    AF = mybir.ActivationFunctionType
    OP = mybir.AluOpType
    AX = mybir.AxisListType

    groups = static["groups"]
    calls = static["calls"]
    calls_of_group = static["calls_of_group"]
    seg_by_chunk = static["seg_by_chunk"]
    seg_list = static["seg_list"]
    tile_call = static["tile_call"]
    tile_tloc = static["tile_tloc"]
    SLOTS = static["SLOTS"]
    NSEG = static["NSEG"]
    MAXSEGC = static["MAXSEGC"]
    seg_chunk = {}
    for i_, sl_ in seg_by_chunk.items():
        for sid in sl_:
            seg_chunk[sid] = i_

    nc = bacc.Bacc("TRN2", target_bir_lowering=False, debug=False,
                   num_devices=NCORE, num_swdge_queues=4)

    x_d = nc.dram_tensor("x_sh", [NPC, F_IN], f32, kind="ExternalInput").ap()
    W1_d = nc.dram_tensor("w1", [F_IN, HID], f32, kind="ExternalInput").ap()
    W2_d = nc.dram_tensor("w2", [HID, CLS], f32, kind="ExternalInput").ap()
    b1_d = nc.dram_tensor("b1c", [128, 2], f32, kind="ExternalInput").ap()
    b2_d = nc.dram_tensor("b2r", [128, CLS], f32, kind="ExternalInput").ap()
    idx_d = nc.dram_tensor("idxs", [128, SLOTS // 16], i16,
                           kind="ExternalInput").ap()
    sdata_d = nc.dram_tensor("sdata", [128, NSEG * 128], f8,
                             kind="ExternalInput").ap()
    selfw_d = nc.dram_tensor("selfwf", [128, NCHK], f32,
                             kind="ExternalInput").ap()
    out_d = nc.dram_tensor("out", [NPC, CLS], f32, kind="ExternalOutput").ap()

    hstage = [[nc.dram_tensor(f"hstage{q}_{pa}", [128, NRQ[q] * 128], f16).ap()
               for q in range(4)] for pa in range(2)]
    ztab = [[nc.dram_tensor(f"ztab{q}_{pa}", [NCORE * 128 * NRQ[q], 128],
                            f16, addr_space="Shared").ap()
             for q in range(4)] for pa in range(2)]

    with tile.TileContext(nc) as tc:
        with (
            tc.tile_pool(name="const", bufs=1) as cp,
            tc.tile_pool(name="resident", bufs=1) as rp,
            tc.tile_pool(name="mlp", bufs=3) as mp,
            tc.tile_pool(name="gb", bufs=18) as gp,
            tc.tile_pool(name="sb", bufs=18) as sp,
            tc.tile_pool(name="sm", bufs=4) as smp,
        ):
            # constants / residents
            idx_t = rp.tile([128, SLOTS // 16], i16)
            nc.sync.dma_start(idx_t[:], idx_d[:])
            selfw_t = rp.tile([128, NCHK], f32)
            nc.sync.dma_start(selfw_t[:], selfw_d[:])
            W1_t = cp.tile([128, HID], f32)
            nc.sync.dma_start(W1_t[:], W1_d[:])
            W2a_t = cp.tile([128, CLS], f32)
            nc.sync.dma_start(W2a_t[:], W2_d[0:128, :])
            W2b_t = cp.tile([128, CLS], f32)
            nc.sync.dma_start(W2b_t[:], W2_d[128:256, :])
            b1_t = cp.tile([128, 2], f32)
            nc.sync.dma_start(b1_t[:], b1_d[:])
            b2_t = cp.tile([128, CLS], f32)
            nc.sync.dma_start(b2_t[:], b2_d[:])
            ident = cp.tile([128, 128], f32)
            make_identity(nc, ident[:])

            ah_t = rp.tile([128, NCHK * 64], f16)
            nc.vector.memset(ah_t[:], 0.0)
            stgA = rp.tile([128, NCHK * 64], f16)
            nc.vector.memset(stgA[:], 0.0)
            stgB = rp.tile([128, NCHK * 64], f16)
            nc.vector.memset(stgB[:], 0.0)

            # ---- MLP: z0 = relu(x@W1+b1)@W2+b2 ----
            with tc.tile_pool(name="psmlp", bufs=2, space="PSUM") as pmp:
                for i in range(NCHK):
                    sz = _chunk_size(i)
                    xt = mp.tile([128, F_IN], f32, tag="xt")
                    nc.sync.dma_start(xt[0:sz, :], x_d[CH * i:CH * i + sz, :])
                    pxT = pmp.tile([128, 128], f32, tag="pmlp")
                    nc.tensor.transpose(pxT[:, 0:sz], xt[0:sz, :],
                                        ident[0:sz, 0:sz])
                    xT = mp.tile([128, 128], f32, tag="xT")
                    nc.scalar.activation(xT[:, 0:sz], pxT[:, 0:sz], AF.Copy)
                    relus = []
                    for h in range(2):
                        ph = pmp.tile([128, 128], f32, tag="pmlp")
                        nc.tensor.matmul(ph[:, 0:sz],
                                         lhsT=W1_t[:, 128 * h:128 * (h + 1)],
                                         rhs=xT[:, 0:sz], start=True,
                                         stop=True)
                        rh = mp.tile([128, 128], f32, tag=f"relu{h}")
                        nc.scalar.activation(rh[:, 0:sz], ph[:, 0:sz],
                                             AF.Relu, bias=b1_t[:, h:h + 1])
                        relus.append(rh)
                    pz = pmp.tile([128, 128], f32, tag="pmlp")
                    for h in range(2):
                        nc.tensor.matmul(pz[0:sz, 0:CLS],
                                         lhsT=relus[h][:, 0:sz],
                                         rhs=(W2a_t if h == 0 else W2b_t)[:],
                                         start=(h == 0), stop=(h == 1))
                    z0 = mp.tile([128, CLS], f32, tag="z0")
                    nc.vector.tensor_tensor(out=z0[0:sz, :],
                                            in0=pz[0:sz, 0:CLS],
                                            in1=b2_t[0:sz, :], op=OP.add)
                    nc.vector.tensor_copy(
                        out=stgA[0:sz, 64 * i:64 * i + CLS], in_=z0[0:sz, :])
                    nc.scalar.mul(ah_t[0:sz, 64 * i:64 * i + CLS],
                                  z0[0:sz, :], ALPHA)

            def stage_group(stg, pa, gi):
                g = groups[gi]
                q = 0
                while g[0] >= QB[q + 1]:
                    q += 1
                i0 = g[0] - QB[q]
                i1 = g[-1] + 1 - QB[q]
                hv = hstage[pa][q][:].rearrange("p (i f) -> p i f", f=128)
                sv = stg[:].rearrange("p (i f) -> p i f", f=64)
                nc.sync.dma_start(hv[:, i0:i1, 0:64],
                                  sv[:, g[0]:g[-1] + 1, :])

            def do_ag(pa, q):
                nc.gpsimd.collective_compute(
                    "AllGather", mybir.AluOpType.bypass,
                    replica_groups=[list(range(NCORE))],
                    ins=[hstage[pa][q][:].opt()],
                    outs=[ztab[pa][q][:].opt()])

            for gi in range(len(groups)):
                stage_group(stgA, 1, gi)
                if gi in AG_AFTER_GROUP:
                    do_ag(1, AG_AFTER_GROUP.index(gi))

            # ---- K propagation steps ----
            stg_prev, stg_new = stgA, stgB
            with tc.tile_pool(name="pschunk", bufs=8, space="PSUM") as psp:
                for k in range(1, K_ITERS + 1):
                    pa = k % 2
                    npa = (k + 1) % 2
                    for gi, grp in enumerate(groups):
                        gtile = {}
                        stile = {}
                        for cid in calls_of_group[gi]:
                            call = calls[cid]
                            nt = call["nt"]
                            r = call["region"]
                            g = gp.tile([128, MAX_CALL_TILES, 128], f16,
                                        tag="g")
                            nc.gpsimd.dma_gather(
                                g[:, 0:nt, :],
                                ztab[pa][r][:],
                                idx_t[:, call["idx_off16"]:
                                      call["idx_off16"] + nt * 8],
                                nt * 128, nt * 128, 128,
                                single_packet=SINGLE_PACKET,
                                queue_num=r,
                            )
                            gtile[cid] = g
                            st = sp.tile([128, MAXSEGC * 128], f8, tag="st")
                            ns = call["ns"]
                            s0 = call["s0"]
                            nc.sync.dma_start(
                                st[:, 0:ns * 128],
                                sdata_d[:, s0 * 128:(s0 + ns) * 128])
                            stile[cid] = st
                        # per-chunk PSUM tiles; matmuls issued TILE-major
                        # (arrival order) with interleaved accum groups so
                        # the PE consumes gathers as they land
                        pstile = {}
                        first = {}
                        last = {}
                        for i in grp:
                            pst = psp.tile([128, 64], f32, tag="ps")
                            pstile[i] = pst
                            segs = seg_by_chunk[i]
                            first[i] = segs[0]
                            last[i] = segs[-1]
                        for cid in calls_of_group[gi]:
                            call = calls[cid]
                            for sj in range(call["ns"]):
                                sid = call["s0"] + sj
                                (t, lo, hi) = seg_list[sid]
                                i = seg_chunk[sid]
                                tloc = tile_tloc[t]
                                nc.tensor.matmul(
                                    pstile[i][:, :],
                                    lhsT=stile[cid][:, 128 * sj:
                                                    128 * (sj + 1)],
                                    rhs=gtile[cid][:, tloc, 0:64],
                                    start=(sid == first[i]),
                                    stop=(sid == last[i]),
                                    skip_group_check=True)
                        for i in grp:
                            sz = _chunk_size(i)
                            ps = pstile[i]
                            # combine: z = ps + selfw*(1-a)*z_prev + a*h
                            t1 = smp.tile([128, 64], f16, tag="t1")
                            nc.vector.tensor_scalar(
                                out=t1[0:sz, :],
                                in0=stg_prev[0:sz, 64 * i:64 * (i + 1)],
                                scalar1=selfw_t[0:sz, i:i + 1],
                                scalar2=None, op0=OP.mult)
                            t2 = smp.tile([128, 64], f16, tag="t2")
                            nc.vector.tensor_tensor(
                                out=t2[0:sz, :], in0=t1[0:sz, :],
                                in1=ah_t[0:sz, 64 * i:64 * (i + 1)],
                                op=OP.add)
                            if k < K_ITERS:
                                nc.vector.tensor_tensor(
                                    out=stg_new[0:sz, 64 * i:64 * (i + 1)],
                                    in0=ps[0:sz, 0:64], in1=t2[0:sz, :],
                                    op=OP.add)
                            else:
                                z = smp.tile([128, CLS], f32, tag="z")
                                nc.vector.tensor_tensor(
                                    out=z[0:sz, :], in0=ps[0:sz, 0:CLS],
                                    in1=t2[0:sz, 0:CLS], op=OP.add)
                                m = smp.tile([128, 1], f32, tag="m")
                                nc.vector.tensor_reduce(
                                    m[0:sz, :], z[0:sz, :], axis=AX.X,
                                    op=OP.max)
                                nm = smp.tile([128, 1], f32, tag="nm")
                                nc.vector.tensor_scalar_mul(
                                    nm[0:sz, :], m[0:sz, :], -1.0)
                                e = smp.tile([128, CLS], f32, tag="e")
                                nc.scalar.activation(e[0:sz, :], z[0:sz, :],
                                                     AF.Exp,
                                                     bias=nm[0:sz, 0:1])
                                s = smp.tile([128, 1], f32, tag="s")
                                nc.vector.tensor_reduce(
                                    s[0:sz, :], e[0:sz, :], axis=AX.X,
                                    op=OP.add)
                                ls = smp.tile([128, 1], f32, tag="ls")
                                nc.scalar.activation(ls[0:sz, :], s[0:sz, :],
                                                     AF.Ln)
                                offs = smp.tile([128, 1], f32, tag="offs")
                                nc.vector.tensor_tensor(
                                    out=offs[0:sz, :], in0=m[0:sz, :],
                                    in1=ls[0:sz, :], op=OP.add)
                                res = smp.tile([128, CLS], f32, tag="res")
                                nc.vector.tensor_scalar(
                                    out=res[0:sz, :], in0=z[0:sz, :],
                                    scalar1=offs[0:sz, 0:1], scalar2=None,
                                    op0=OP.subtract)
                                nc.sync.dma_start(
                                    out_d[CH * i:CH * i + sz, :],
                                    res[0:sz, :])
                        if k < K_ITERS:
                            stage_group(stg_new, npa, gi)
                            if gi in AG_AFTER_GROUP:
                                do_ag(npa, AG_AFTER_GROUP.index(gi))
                    if k < K_ITERS:
                        stg_prev, stg_new = stg_new, stg_prev

    nc.compile()
    return nc


def kernel(x, edge_index, W1, b1, W2, b2):
    global LAST_EXEC_NS, LAST_SCOPES
    from concourse import bass_utils

    x = np.asarray(x, np.float32)
    ei = np.asarray(edge_index)
    W1 = np.asarray(W1, np.float32)
    b1 = np.asarray(b1, np.float32)
    W2 = np.asarray(W2, np.float32)
    b2 = np.asarray(b2, np.float32)

    static, per_core = _preprocess(ei)
    nc = _build(static)

    b1c = np.stack([b1[0:128], b1[128:256]], axis=1).astype(np.float32)
    b1c = np.ascontiguousarray(b1c)
    b2r = np.ascontiguousarray(np.tile(b2[None, :], (128, 1)).astype(np.float32))

    in_maps = []
    for c in range(NCORE):
        in_maps.append({
            "x_sh": np.ascontiguousarray(x[c * NPC:(c + 1) * NPC]),
            "w1": W1, "w2": W2, "b1c": b1c, "b2r": b2r,
            "idxs": np.ascontiguousarray(per_core["idx_sb"][c]),
            "sdata": np.ascontiguousarray(per_core["sdata"][c]),
            "selfwf": np.ascontiguousarray(per_core["selfw_sb"][c]),
        })

    if TRACE:
        _install_ntff_hook()
    res = bass_utils.run_bass_kernel_spmd(
        nc, in_maps, core_ids=list(range(NCORE)), trace=TRACE)
    LAST_EXEC_NS = res.exec_time_ns
    LAST_SCOPES = res.per_core_scope_times

    out = np.concatenate([res.results[c]["out"] for c in range(NCORE)], axis=0)
    return out.astype(np.float32)
